# revision 1
# baseline (speedup 1.0000x reference)
"""Trainium2 Bass kernel for CollaborationGNNWithFeatures.

2-layer GraphSAGE (mean aggr) + edge-feature MLP + link predictor over
1M prediction edges, on 8 NeuronCores.

Sharding: message edges sorted by dst and sharded by dst-node range
(12500 nodes/core) -> segment-sum is core-local (no all-reduce); the
per-layer node table needed by the next layer's gathers is exchanged
with a small AllGather ([12500,64] f32 per rank). Prediction edges are
split evenly (125k/core) in original order.

segment_sum on device: edges in 128-dst windows; per 128-edge block an
indirect-DMA row gather (msg [128e, C]) and a fused one-hot
(is_equal x 1/deg, so mean is folded in) feed one PE matmul
psum[C, 128dst] += msg.T @ onehot, accumulated over the window's blocks.
"""
import numpy as np

import concourse.bass as bass
import concourse.bacc as bacc
import concourse.mybir as mybir
import concourse.tile as tile

N_CORES = 8
N = 100000
E = 1600000
P_EDGES = 1000000
DIN = 128
H = 128
DOUT = 64
EIN = 32
EPS = 1e-5

NPC = N // N_CORES          # nodes per core: 12500
WIN = 128                   # dst window width
NWIN = (NPC + WIN - 1) // WIN   # 98 windows/core
PPC = P_EDGES // N_CORES    # pred edges per core: 125000
SB = 4                      # pred blocks per superblock
NPB = ((PPC + 127) // 128 + SB - 1) // SB * SB   # 980 pred blocks
NSB = NPB // SB             # 245 superblocks

F32 = mybir.dt.float32
I32 = mybir.dt.int32


def _prep_host(inputs):
    """All index bookkeeping / layout prep. Returns (in_maps, nblk, meta)."""
    x = np.asarray(inputs["x"], np.float32)
    ei = np.asarray(inputs["edge_index"])
    pei = np.asarray(inputs["pred_edge_index"])
    ef = np.asarray(inputs["edge_features"], np.float32)

    src = ei[0].astype(np.int64)
    dst = ei[1].astype(np.int64)

    deg = np.bincount(dst, minlength=N).astype(np.float32)
    recip = 1.0 / np.maximum(deg, 1.0)

    order = np.argsort(dst, kind="stable")
    s_src = src[order].astype(np.int32)
    s_dst = dst[order].astype(np.int64)

    # per (core, window) edge lists
    core_of = s_dst // NPC
    win_of = (s_dst % NPC) // WIN
    # counts[core, win]
    counts = np.zeros((N_CORES, NWIN), np.int64)
    np.add.at(counts, (core_of, win_of), 1)
    nblk = np.maximum((counts + 127) // 128, 1).max(axis=0)  # shared blocks/window
    NB = int(nblk.sum())

    # block -> window map (shared across cores)
    blk_win = np.repeat(np.arange(NWIN), nblk)

    # pack per-core edge data into [128, NB] arrays
    srcA = np.zeros((N_CORES, 128, NB), np.int32)
    drelA = np.full((N_CORES, 128, NB), -1.0, np.float32)
    wA = np.zeros((N_CORES, 128, NB), np.float32)

    # edges are sorted by dst => grouped by (core, win) in order
    bounds = np.zeros((N_CORES, NWIN + 1), np.int64)
    for c in range(N_CORES):
        for w in range(NWIN):
            bounds[c, w + 1] = bounds[c, w] + counts[c, w]
    core_starts = np.concatenate([[0], np.cumsum(counts.sum(axis=1))])
    blk_starts = np.concatenate([[0], np.cumsum(nblk)])
    for c in range(N_CORES):
        base = core_starts[c]
        for w in range(NWIN):
            cnt = int(counts[c, w])
            e0 = base + bounds[c, w]
            b0 = int(blk_starts[w])
            es = s_src[e0:e0 + cnt]
            ed = (s_dst[e0:e0 + cnt] % NPC - w * WIN).astype(np.float32)
            ew = recip[s_dst[e0:e0 + cnt]]
            nfull = cnt // 128
            k = nfull * 128
            A = srcA[c]
            D_ = drelA[c]
            W_ = wA[c]
            if nfull:
                A[:, b0:b0 + nfull] = es[:k].reshape(nfull, 128).T
                D_[:, b0:b0 + nfull] = ed[:k].reshape(nfull, 128).T
                W_[:, b0:b0 + nfull] = ew[:k].reshape(nfull, 128).T
            r = cnt - k
            if r:
                A[:r, b0 + nfull] = es[k:]
                D_[:r, b0 + nfull] = ed[k:]
                W_[:r, b0 + nfull] = ew[k:]

    # pred edges: even split, original order, pad to NPB blocks
    ps = pei[0].astype(np.int32)
    pd = pei[1].astype(np.int32)
    sA = np.zeros((N_CORES, 128, NPB), np.int32)
    dA = np.zeros((N_CORES, 128, NPB), np.int32)
    efA = np.zeros((N_CORES, 32, NPB * 128), np.float32)
    for c in range(N_CORES):
        e0 = c * PPC
        padded_s = np.zeros(NPB * 128, np.int32)
        padded_d = np.zeros(NPB * 128, np.int32)
        padded_s[:PPC] = ps[e0:e0 + PPC]
        padded_d[:PPC] = pd[e0:e0 + PPC]
        sA[c] = padded_s.reshape(NPB, 128).T
        dA[c] = padded_d.reshape(NPB, 128).T
        efA[c, :, :PPC] = ef[e0:e0 + PPC].T

    # weights (shared), pre-transposed for lhsT use
    g = inputs
    f32 = lambda a: np.ascontiguousarray(np.asarray(a, np.float32))
    col = lambda a: f32(a).reshape(-1, 1)
    s1 = f32(g["bn1_g"]) / np.sqrt(f32(g["bn1_v"]) + EPS)
    t1 = (f32(g["sage1_bl"]) - f32(g["bn1_m"])) * s1 + f32(g["bn1_b"])
    es_ = f32(g["ebn_g"]) / np.sqrt(f32(g["ebn_v"]) + EPS)
    et_ = (f32(g["edge_b1"]) - f32(g["ebn_m"])) * es_ + f32(g["ebn_b"])
    ps1 = f32(g["pbn1_g"]) / np.sqrt(f32(g["pbn1_v"]) + EPS)
    pt1 = (f32(g["p_b1"]) - f32(g["pbn1_m"])) * ps1 + f32(g["pbn1_b"])
    ps2 = f32(g["pbn2_g"]) / np.sqrt(f32(g["pbn2_v"]) + EPS)
    pt2 = (f32(g["p_b2"]) - f32(g["pbn2_m"])) * ps2 + f32(g["pbn2_b"])

    weights = {
        "Wl1T": f32(g["sage1_Wl"].T), "Wr1T": f32(g["sage1_Wr"].T),
        "s1": col(s1), "t1": col(t1),
        "Wl2T": f32(g["sage2_Wl"].T), "Wr2T": f32(g["sage2_Wr"].T),
        "bl2": col(g["sage2_bl"]),
        "eW1T": f32(g["edge_W1"].T), "es": col(es_), "et": col(et_),
        "eW2T": f32(g["edge_W2"].T), "eb2": col(g["edge_b2"]),
        "W1abT": f32(g["p_W1"][:, :2 * DOUT].T),
        "W1cT": f32(g["p_W1"][:, 2 * DOUT:].T),
        "ps1": col(ps1), "pt1": col(pt1),
        "W2pT": f32(g["p_W2"].T), "ps2": col(ps2), "pt2": col(pt2),
        "W3pT": f32(g["p_W3"].T), "pb3": col(g["p_b3"]),
        "iotaF": np.tile(np.arange(128, dtype=np.float32), (128, 1)),
        "ident": np.eye(128, dtype=np.float32),
    }

    in_maps = []
    for c in range(N_CORES):
        m = {
            "x_full": x,
            "xT_loc": np.ascontiguousarray(x[c * NPC:(c + 1) * NPC].T),
            "srcI": srcA[c], "drel": drelA[c], "wgt": wA[c],
            "psI": sA[c], "pdI": dA[c], "efT": efA[c],
        }
        m.update(weights)
        in_maps.append(m)
    return in_maps, nblk, blk_starts


def _build(nblk, blk_starts):
    NB = int(nblk.sum())
    nc = bacc.Bacc("TRN2", target_bir_lowering=False, debug=False,
                   num_devices=N_CORES)

    x_full = nc.dram_tensor("x_full", [N, DIN], F32, kind="ExternalInput")
    xT_loc = nc.dram_tensor("xT_loc", [DIN, NPC], F32, kind="ExternalInput")
    srcI = nc.dram_tensor("srcI", [128, NB], I32, kind="ExternalInput")
    drel = nc.dram_tensor("drel", [128, NB], F32, kind="ExternalInput")
    wgt = nc.dram_tensor("wgt", [128, NB], F32, kind="ExternalInput")
    psI = nc.dram_tensor("psI", [128, NPB], I32, kind="ExternalInput")
    pdI = nc.dram_tensor("pdI", [128, NPB], I32, kind="ExternalInput")
    efT = nc.dram_tensor("efT", [32, NPB * 128], F32, kind="ExternalInput")

    wt = {}
    for name, shape in [
        ("Wl1T", [DIN, H]), ("Wr1T", [DIN, H]), ("s1", [H, 1]), ("t1", [H, 1]),
        ("Wl2T", [H, DOUT]), ("Wr2T", [H, DOUT]), ("bl2", [DOUT, 1]),
        ("eW1T", [EIN, 64]), ("es", [64, 1]), ("et", [64, 1]),
        ("eW2T", [64, 32]), ("eb2", [32, 1]),
        ("W1abT", [128, 128]), ("W1cT", [32, 128]),
        ("ps1", [128, 1]), ("pt1", [128, 1]),
        ("W2pT", [128, 64]), ("ps2", [64, 1]), ("pt2", [64, 1]),
        ("W3pT", [64, 1]), ("pb3", [1, 1]),
        ("iotaF", [128, 128]), ("ident", [128, 128]),
    ]:
        wt[name] = nc.dram_tensor(name, shape, F32, kind="ExternalInput")

    out = nc.dram_tensor("out", [NPB * 128], F32, kind="ExternalOutput")

    # dense chunking of the 12500-node range
    chunks = []
    c0 = 0
    while c0 < NPC:
        cw = min(512, NPC - c0)
        chunks.append((c0, cw))
        c0 += cw

    with tile.TileContext(nc) as tc:
        with (
            tc.tile_pool(name="const", bufs=1) as constp,
            tc.tile_pool(name="meta", bufs=1) as metap,
            tc.tile_pool(name="agg", bufs=1) as aggp,
            tc.tile_pool(name="msgs", bufs=4) as msgs,
            tc.tile_pool(name="ohp", bufs=4) as ohp,
            tc.tile_pool(name="wpsum", bufs=2, space="PSUM") as wpsum,
            tc.tile_pool(name="dpsum", bufs=2, space="PSUM") as dpsum,
            tc.tile_pool(name="tpsum", bufs=2, space="PSUM") as tpsum,
            tc.tile_pool(name="work", bufs=2) as work,
            tc.tile_pool(name="nodew", bufs=2) as nodew,
        ):
            # ---- load constants / weights / metadata ----
            W = {}
            for name in wt:
                W[name] = constp.tile(list(wt[name].shape), F32, tag=name, name=f"w_{name}")
                nc.sync.dma_start(W[name][:], wt[name][:])
            srcT = metap.tile([128, NB], I32, tag="srcT")
            nc.sync.dma_start(srcT[:], srcI[:])
            drelT = metap.tile([128, NB], F32, tag="drelT")
            nc.sync.dma_start(drelT[:], drel[:])
            wgtT = metap.tile([128, NB], F32, tag="wgtT")
            nc.sync.dma_start(wgtT[:], wgt[:])
            psT = metap.tile([128, NPB], I32, tag="psT")
            nc.sync.dma_start(psT[:], psI[:])
            pdT = metap.tile([128, NPB], I32, tag="pdT")
            nc.sync.dma_start(pdT[:], pdI[:])

            aggT = aggp.tile([128, NPC], F32, tag="aggT")      # L1 sums (ch-major)
            aggT2 = aggp.tile([64, NPC], F32, tag="aggT2")     # L2 sums

            h1T_d = nc.dram_tensor("h1T_d", [DIN, NPC], F32, kind="Internal")
            g2_loc = nc.dram_tensor("g2_loc", [NPC, DOUT], F32, kind="Internal")
            g2_full = nc.dram_tensor("g2_full", [N, DOUT], F32, kind="Internal",
                                     addr_space="Shared")
            z_loc = nc.dram_tensor("z_loc", [NPC, DOUT], F32, kind="Internal")
            z_full = nc.dram_tensor("z_full", [N, DOUT], F32, kind="Internal",
                                    addr_space="Shared")

            def seg_layer(table_ap, ch, dst_agg):
                """Windowed segment-sum: dst_agg[:ch, :] accumulates
                (sum_e msg[e]*w[e]) per dst, transposed (ch-major)."""
                for w in range(NWIN):
                    w0 = w * WIN
                    wlen = min(WIN, NPC - w0)
                    b0, b1 = int(blk_starts[w]), int(blk_starts[w + 1])
                    pt = wpsum.tile([ch, 128], F32, tag="segp")
                    for b in range(b0, b1):
                        m = msgs.tile([128, ch], F32, tag="m")
                        nc.gpsimd.indirect_dma_start(
                            out=m[:], out_offset=None, in_=table_ap,
                            in_offset=bass.IndirectOffsetOnAxis(
                                ap=srcT[:, b:b + 1], axis=0),
                        )
                        oh = ohp.tile([128, 128], F32, tag="oh")
                        nc.vector.tensor_scalar(
                            out=oh[:], in0=W["iotaF"][:],
                            scalar1=drelT[:, b:b + 1],
                            scalar2=wgtT[:, b:b + 1],
                            op0=mybir.AluOpType.is_equal,
                            op1=mybir.AluOpType.mult,
                        )
                        nc.tensor.matmul(pt[:], m[:], oh[:],
                                         start=(b == b0), stop=(b == b1 - 1))
                    nc.scalar.copy(dst_agg[:ch, w0:w0 + wlen], pt[:, :wlen])

            # ================= layer 1 =================
            seg_layer(x_full[:], DIN, aggT)

            for (c0, cw) in chunks:
                xt = nodew.tile([128, 512], F32, tag="xt")
                nc.sync.dma_start(xt[:, :cw], xT_loc[:, c0:c0 + cw])
                d1 = dpsum.tile([128, 512], F32, tag="big")
                nc.tensor.matmul(d1[:, :cw], W["Wl1T"][:], aggT[:, c0:c0 + cw],
                                 start=True, stop=False)
                nc.tensor.matmul(d1[:, :cw], W["Wr1T"][:], xt[:, :cw],
                                 start=False, stop=True)
                h1t = work.tile([128, 512], F32, tag="h1t")
                nc.scalar.activation(h1t[:, :cw], d1[:, :cw],
                                     mybir.ActivationFunctionType.Relu,
                                     bias=W["t1"][:], scale=W["s1"][:])
                nc.sync.dma_start(h1T_d[:, c0:c0 + cw], h1t[:, :cw])
                # g2 = (h1 @ Wl2.T) -> node-major table for layer-2 gathers
                g2p = dpsum.tile([64, 512], F32, tag="small")
                nc.tensor.matmul(g2p[:, :cw], W["Wl2T"][:], h1t[:, :cw],
                                 start=True, stop=True)
                g2s = work.tile([64, 512], F32, tag="g2s")
                nc.vector.tensor_copy(g2s[:, :cw], g2p[:, :cw])
                for j in range(0, cw, 128):
                    jw = min(128, cw - j)
                    tp = tpsum.tile([128, 64], F32, tag="tp")
                    nc.tensor.transpose(tp[:jw, :], g2s[:, j:j + jw],
                                        W["ident"][:64, :64])
                    tn = work.tile([128, 64], F32, tag="tn")
                    nc.vector.tensor_copy(tn[:jw, :], tp[:jw, :])
                    nc.sync.dma_start(g2_loc[c0 + j:c0 + j + jw, :], tn[:jw, :])

            nc.gpsimd.collective_compute(
                "AllGather", mybir.AluOpType.bypass,
                ins=[g2_loc[:]], outs=[g2_full[:]],
                replica_groups=[list(range(N_CORES))],
            )

            # ================= layer 2 =================
            seg_layer(g2_full[:], DOUT, aggT2)

            for (c0, cw) in chunks:
                h1t = nodew.tile([128, 512], F32, tag="xt")
                nc.sync.dma_start(h1t[:, :cw], h1T_d[:, c0:c0 + cw])
                zp = dpsum.tile([64, 512], F32, tag="small")
                nc.tensor.matmul(zp[:, :cw], W["Wr2T"][:], h1t[:, :cw],
                                 start=True, stop=True)
                zs = work.tile([64, 512], F32, tag="g2s")
                nc.vector.tensor_add(zs[:, :cw], zp[:, :cw],
                                     aggT2[:, c0:c0 + cw])
                zb = work.tile([64, 512], F32, tag="zb")
                nc.vector.tensor_scalar_add(zb[:, :cw], zs[:, :cw],
                                            W["bl2"][:])
                for j in range(0, cw, 128):
                    jw = min(128, cw - j)
                    tp = tpsum.tile([128, 64], F32, tag="tp")
                    nc.tensor.transpose(tp[:jw, :], zb[:, j:j + jw],
                                        W["ident"][:64, :64])
                    tn = work.tile([128, 64], F32, tag="tn")
                    nc.vector.tensor_copy(tn[:jw, :], tp[:jw, :])
                    nc.sync.dma_start(z_loc[c0 + j:c0 + j + jw, :], tn[:jw, :])

            nc.gpsimd.collective_compute(
                "AllGather", mybir.AluOpType.bypass,
                ins=[z_loc[:]], outs=[z_full[:]],
                replica_groups=[list(range(N_CORES))],
            )

            # ================= predictor =================
            for sb in range(NSB):
                combT = work.tile([128, 512], F32, tag="combT")
                for i in range(SB):
                    b = sb * SB + i
                    cb = msgs.tile([128, 128], F32, tag="cb")
                    nc.gpsimd.indirect_dma_start(
                        out=cb[:, 0:64], out_offset=None, in_=z_full[:],
                        in_offset=bass.IndirectOffsetOnAxis(
                            ap=psT[:, b:b + 1], axis=0),
                    )
                    nc.gpsimd.indirect_dma_start(
                        out=cb[:, 64:128], out_offset=None, in_=z_full[:],
                        in_offset=bass.IndirectOffsetOnAxis(
                            ap=pdT[:, b:b + 1], axis=0),
                    )
                    ctp = tpsum.tile([128, 128], F32, tag="tp")
                    nc.tensor.transpose(ctp[:], cb[:], W["ident"][:])
                    nc.vector.tensor_copy(combT[:, i * 128:(i + 1) * 128],
                                          ctp[:])
                eft = nodew.tile([32, 512], F32, tag="eft")
                nc.sync.dma_start(eft[:],
                                  efT[:, sb * 512:(sb + 1) * 512])
                em1 = dpsum.tile([64, 512], F32, tag="small")
                nc.tensor.matmul(em1[:], W["eW1T"][:], eft[:],
                                 start=True, stop=True)
                em1s = work.tile([64, 512], F32, tag="em1s")
                nc.scalar.activation(em1s[:], em1[:],
                                     mybir.ActivationFunctionType.Relu,
                                     bias=W["et"][:], scale=W["es"][:])
                em2 = dpsum.tile([32, 512], F32, tag="small")
                nc.tensor.matmul(em2[:], W["eW2T"][:], em1s[:],
                                 start=True, stop=True)
                em2s = work.tile([32, 512], F32, tag="em2s")
                nc.vector.tensor_scalar_add(em2s[:], em2[:], W["eb2"][:])
                u1 = dpsum.tile([128, 512], F32, tag="big")
                nc.tensor.matmul(u1[:], W["W1abT"][:], combT[:],
                                 start=True, stop=False)
                nc.tensor.matmul(u1[:], W["W1cT"][:], em2s[:],
                                 start=False, stop=True)
                u1s = work.tile([128, 512], F32, tag="u1s")
                nc.scalar.activation(u1s[:], u1[:],
                                     mybir.ActivationFunctionType.Relu,
                                     bias=W["pt1"][:], scale=W["ps1"][:])
                u2 = dpsum.tile([64, 512], F32, tag="small")
                nc.tensor.matmul(u2[:], W["W2pT"][:], u1s[:],
                                 start=True, stop=True)
                u2s = work.tile([64, 512], F32, tag="u2s")
                nc.scalar.activation(u2s[:], u2[:],
                                     mybir.ActivationFunctionType.Relu,
                                     bias=W["pt2"][:], scale=W["ps2"][:])
                uo = tpsum.tile([1, 512], F32, tag="tp")
                nc.tensor.matmul(uo[:], W["W3pT"][:], u2s[:],
                                 start=True, stop=True)
                uos = work.tile([1, 512], F32, tag="uos")
                nc.vector.tensor_scalar_add(uos[:], uo[:], W["pb3"][:])
                nc.sync.dma_start(
                    out[sb * 512:(sb + 1) * 512].rearrange("(p f) -> p f", p=1),
                    uos[:])

    nc.compile()
    return nc


def kernel(**inputs):
    from concourse import bass_utils

    in_maps, nblk, blk_starts = _prep_host(inputs)
    nc = _build(nblk, blk_starts)
    res = bass_utils.run_bass_kernel_spmd(
        nc, in_maps, core_ids=list(range(N_CORES)))
    outs = []
    for c in range(N_CORES):
        outs.append(res.results[c]["out"][:PPC])
    return np.concatenate(outs).astype(np.float32)



# revision 50
# speedup vs baseline: 8.8183x; 8.8183x over previous
"""Trainium2 Bass kernel for CollaborationGNNWithFeatures.

2-layer GraphSAGE (mean aggr) + edge-feature MLP + link predictor over
1M prediction edges, on 8 NeuronCores.

Sharding: message edges sorted by dst and sharded by dst-node range
(12500 nodes/core) -> segment-sum is core-local (no all-reduce); the
per-layer node table needed by the next layer's gathers is exchanged
with a small AllGather ([12500,64] f32 per rank). Prediction edges are
split evenly (125k/core) in original order.

segment_sum on device: edges in 128-dst windows; per 128-edge block an
indirect-DMA row gather (msg [128e, C]) and a fused one-hot
(is_equal x 1/deg, so mean is folded in) feed one PE matmul
psum[C, 128dst] += msg.T @ onehot, accumulated over the window's blocks.
"""
import numpy as np

import concourse.bass as bass
import concourse.bacc as bacc
import concourse.mybir as mybir
import concourse.tile as tile

N_CORES = 8
N = 100000
E = 1600000
P_EDGES = 1000000
DIN = 128
H = 128
DOUT = 64
EIN = 32
EPS = 1e-5

NPC = N // N_CORES          # nodes per core: 12500
WIN = 128                   # dst window width
NWIN = (NPC + WIN - 1) // WIN   # 98 windows/core
PPC = P_EDGES // N_CORES    # pred edges per core: 125000
SB = 4                      # pred blocks per superblock
NPB = ((PPC + 127) // 128 + SB - 1) // SB * SB   # 980 pred blocks
NSB = NPB // SB             # 245 superblocks

F32 = mybir.dt.float32
I32 = mybir.dt.int32


def _prep_host(inputs):
    """All index bookkeeping / layout prep. Returns (in_maps, nblk, meta)."""
    x = np.asarray(inputs["x"], np.float32)
    ei = np.asarray(inputs["edge_index"])
    pei = np.asarray(inputs["pred_edge_index"])
    ef = np.asarray(inputs["edge_features"], np.float32)

    src = ei[0].astype(np.int64)
    dst = ei[1].astype(np.int64)

    deg = np.bincount(dst, minlength=N).astype(np.float32)
    recip = 1.0 / np.maximum(deg, 1.0)

    order = np.argsort(dst, kind="stable")
    s_src = src[order].astype(np.int32)
    s_dst = dst[order].astype(np.int64)

    # per (core, window) edge lists
    core_of = s_dst // NPC
    win_of = (s_dst % NPC) // WIN
    # counts[core, win]
    counts = np.zeros((N_CORES, NWIN), np.int64)
    np.add.at(counts, (core_of, win_of), 1)
    nblk = np.maximum((counts + 127) // 128, 1).max(axis=0)  # shared blocks/window
    NB = int(nblk.sum())

    # block -> window map (shared across cores)
    blk_win = np.repeat(np.arange(NWIN), nblk)

    # pack per-core edge data into [128, NB] arrays
    srcA = np.zeros((N_CORES, 128, NB), np.int32)
    drelA = np.full((N_CORES, 128, NB), -1.0, np.float32)
    wA = np.zeros((N_CORES, 128, NB), np.float32)

    # edges are sorted by dst => grouped by (core, win) in order
    bounds = np.zeros((N_CORES, NWIN + 1), np.int64)
    for c in range(N_CORES):
        for w in range(NWIN):
            bounds[c, w + 1] = bounds[c, w] + counts[c, w]
    core_starts = np.concatenate([[0], np.cumsum(counts.sum(axis=1))])
    blk_starts = np.concatenate([[0], np.cumsum(nblk)])
    for c in range(N_CORES):
        base = core_starts[c]
        for w in range(NWIN):
            cnt = int(counts[c, w])
            e0 = base + bounds[c, w]
            b0 = int(blk_starts[w])
            es = s_src[e0:e0 + cnt]
            ed = (s_dst[e0:e0 + cnt] % NPC - w * WIN).astype(np.float32)
            ew = recip[s_dst[e0:e0 + cnt]]
            nfull = cnt // 128
            k = nfull * 128
            A = srcA[c]
            D_ = drelA[c]
            W_ = wA[c]
            if nfull:
                A[:, b0:b0 + nfull] = es[:k].reshape(nfull, 128).T
                D_[:, b0:b0 + nfull] = ed[:k].reshape(nfull, 128).T
                W_[:, b0:b0 + nfull] = ew[:k].reshape(nfull, 128).T
            r = cnt - k
            if r:
                A[:r, b0 + nfull] = es[k:]
                D_[:r, b0 + nfull] = ed[k:]
                W_[:r, b0 + nfull] = ew[k:]

    # pred edges: even split, original order, pad to NPB blocks
    ps = pei[0].astype(np.int32)
    pd = pei[1].astype(np.int32)
    sA = np.zeros((N_CORES, 128, NPB), np.int32)
    dA = np.zeros((N_CORES, 128, NPB), np.int32)
    efA = np.zeros((N_CORES, 32, NPB * 128), np.float32)
    for c in range(N_CORES):
        e0 = c * PPC
        padded_s = np.zeros(NPB * 128, np.int32)
        padded_d = np.zeros(NPB * 128, np.int32)
        padded_s[:PPC] = ps[e0:e0 + PPC]
        padded_d[:PPC] = pd[e0:e0 + PPC]
        sA[c] = padded_s.reshape(NPB, 128).T
        dA[c] = padded_d.reshape(NPB, 128).T
        efA[c, :, :PPC] = ef[e0:e0 + PPC].T

    # weights (shared), pre-transposed for lhsT use
    g = inputs
    f32 = lambda a: np.ascontiguousarray(np.asarray(a, np.float32))
    col = lambda a: f32(a).reshape(-1, 1)
    s1 = f32(g["bn1_g"]) / np.sqrt(f32(g["bn1_v"]) + EPS)
    t1 = (f32(g["sage1_bl"]) - f32(g["bn1_m"])) * s1 + f32(g["bn1_b"])
    es_ = f32(g["ebn_g"]) / np.sqrt(f32(g["ebn_v"]) + EPS)
    et_ = (f32(g["edge_b1"]) - f32(g["ebn_m"])) * es_ + f32(g["ebn_b"])
    ps1 = f32(g["pbn1_g"]) / np.sqrt(f32(g["pbn1_v"]) + EPS)
    pt1 = (f32(g["p_b1"]) - f32(g["pbn1_m"])) * ps1 + f32(g["pbn1_b"])
    ps2 = f32(g["pbn2_g"]) / np.sqrt(f32(g["pbn2_v"]) + EPS)
    pt2 = (f32(g["p_b2"]) - f32(g["pbn2_m"])) * ps2 + f32(g["pbn2_b"])

    weights = {
        "Wl1T": f32(g["sage1_Wl"].T), "Wr1T": f32(g["sage1_Wr"].T),
        "s1": col(s1), "t1": col(t1),
        "Wl2T": f32(g["sage2_Wl"].T), "Wr2T": f32(g["sage2_Wr"].T),
        "bl2": col(g["sage2_bl"]),
        "eW1T": f32(g["edge_W1"].T), "es": col(es_), "et": col(et_),
        "eW2T": f32(g["edge_W2"].T), "eb2": col(g["edge_b2"]),
        "W1abT": f32(g["p_W1"][:, :2 * DOUT].T),
        "W1cT": f32(g["p_W1"][:, 2 * DOUT:].T),
        "ps1": col(ps1), "pt1": col(pt1),
        "W2pT": f32(g["p_W2"].T), "ps2": col(ps2), "pt2": col(pt2),
        "W3pT": f32(g["p_W3"].T), "pb3": col(g["p_b3"]),
        "iotaF": np.tile(np.arange(128, dtype=np.float32), (128, 1)),
        "ident": np.eye(128, dtype=np.float32),
    }

    in_maps = []
    for c in range(N_CORES):
        m = {
            "x_full": x,
            "xT_loc": np.ascontiguousarray(x[c * NPC:(c + 1) * NPC].T),
            "srcI": srcA[c], "drel": drelA[c], "wgt": wA[c],
            "psI": sA[c], "pdI": dA[c], "efT": efA[c],
        }
        m.update(weights)
        in_maps.append(m)
    return in_maps, nblk, blk_starts


def _build(nblk, blk_starts):
    NB = int(nblk.sum())
    nc = bacc.Bacc("TRN2", target_bir_lowering=False, debug=False,
                   num_devices=N_CORES)

    x_full = nc.dram_tensor("x_full", [N, DIN], F32, kind="ExternalInput")
    xT_loc = nc.dram_tensor("xT_loc", [DIN, NPC], F32, kind="ExternalInput")
    srcI = nc.dram_tensor("srcI", [128, NB], I32, kind="ExternalInput")
    drel = nc.dram_tensor("drel", [128, NB], F32, kind="ExternalInput")
    wgt = nc.dram_tensor("wgt", [128, NB], F32, kind="ExternalInput")
    psI = nc.dram_tensor("psI", [128, NPB], I32, kind="ExternalInput")
    pdI = nc.dram_tensor("pdI", [128, NPB], I32, kind="ExternalInput")
    efT = nc.dram_tensor("efT", [32, NPB * 128], F32, kind="ExternalInput")

    wt = {}
    for name, shape in [
        ("Wl1T", [DIN, H]), ("Wr1T", [DIN, H]), ("s1", [H, 1]), ("t1", [H, 1]),
        ("Wl2T", [H, DOUT]), ("Wr2T", [H, DOUT]), ("bl2", [DOUT, 1]),
        ("eW1T", [EIN, 64]), ("es", [64, 1]), ("et", [64, 1]),
        ("eW2T", [64, 32]), ("eb2", [32, 1]),
        ("W1abT", [128, 128]), ("W1cT", [32, 128]),
        ("ps1", [128, 1]), ("pt1", [128, 1]),
        ("W2pT", [128, 64]), ("ps2", [64, 1]), ("pt2", [64, 1]),
        ("W3pT", [64, 1]), ("pb3", [1, 1]),
        ("iotaF", [128, 128]), ("ident", [128, 128]),
    ]:
        wt[name] = nc.dram_tensor(name, shape, F32, kind="ExternalInput")

    out = nc.dram_tensor("out", [NPB * 128], F32, kind="ExternalOutput")

    # dense chunking of the 12500-node range
    chunks = []
    c0 = 0
    while c0 < NPC:
        cw = min(512, NPC - c0)
        chunks.append((c0, cw))
        c0 += cw

    with tile.TileContext(nc) as tc:
        with (
            tc.tile_pool(name="const", bufs=1) as constp,
            tc.tile_pool(name="meta", bufs=1) as metap,
            tc.tile_pool(name="agg", bufs=1) as aggp,
            tc.tile_pool(name="msgs", bufs=4) as msgs,
            tc.tile_pool(name="ohp", bufs=4) as ohp,
            tc.tile_pool(name="wpsum", bufs=2, space="PSUM") as wpsum,
            tc.tile_pool(name="dpsum", bufs=2, space="PSUM") as dpsum,
            tc.tile_pool(name="tpsum", bufs=2, space="PSUM") as tpsum,
            tc.tile_pool(name="work", bufs=2) as work,
            tc.tile_pool(name="nodew", bufs=2) as nodew,
        ):
            # ---- load constants / weights / metadata ----
            W = {}
            for name in wt:
                W[name] = constp.tile(list(wt[name].shape), F32, tag=name, name=f"w_{name}")
                nc.sync.dma_start(W[name][:], wt[name][:])
            srcT = metap.tile([128, NB], I32, tag="srcT")
            nc.sync.dma_start(srcT[:], srcI[:])
            drelT = metap.tile([128, NB], F32, tag="drelT")
            nc.sync.dma_start(drelT[:], drel[:])
            wgtT = metap.tile([128, NB], F32, tag="wgtT")
            nc.sync.dma_start(wgtT[:], wgt[:])
            psT = metap.tile([128, NPB], I32, tag="psT")
            nc.sync.dma_start(psT[:], psI[:])
            pdT = metap.tile([128, NPB], I32, tag="pdT")
            nc.sync.dma_start(pdT[:], pdI[:])

            aggT = aggp.tile([128, NPC], F32, tag="aggT")      # L1 sums (ch-major)
            aggT2 = aggp.tile([64, NPC], F32, tag="aggT2")     # L2 sums

            h1T_d = nc.dram_tensor("h1T_d", [DIN, NPC], F32, kind="Internal")
            g2_loc = nc.dram_tensor("g2_loc", [NPC, DOUT], F32, kind="Internal")
            g2_full = nc.dram_tensor("g2_full", [N, DOUT], F32, kind="Internal",
                                     addr_space="Shared")
            z_loc = nc.dram_tensor("z_loc", [NPC, DOUT], F32, kind="Internal")
            z_full = nc.dram_tensor("z_full", [N, DOUT], F32, kind="Internal",
                                    addr_space="Shared")

            def seg_layer(table_ap, ch, dst_agg):
                """Windowed segment-sum: dst_agg[:ch, :] accumulates
                (sum_e msg[e]*w[e]) per dst, transposed (ch-major)."""
                for w in range(NWIN):
                    w0 = w * WIN
                    wlen = min(WIN, NPC - w0)
                    b0, b1 = int(blk_starts[w]), int(blk_starts[w + 1])
                    pt = wpsum.tile([ch, 128], F32, tag="segp")
                    for b in range(b0, b1):
                        m = msgs.tile([128, ch], F32, tag="m")
                        nc.gpsimd.indirect_dma_start(
                            out=m[:], out_offset=None, in_=table_ap,
                            in_offset=bass.IndirectOffsetOnAxis(
                                ap=srcT[:, b:b + 1], axis=0),
                        )
                        oh = ohp.tile([128, 128], F32, tag="oh")
                        nc.vector.tensor_scalar(
                            out=oh[:], in0=W["iotaF"][:],
                            scalar1=drelT[:, b:b + 1],
                            scalar2=wgtT[:, b:b + 1],
                            op0=mybir.AluOpType.is_equal,
                            op1=mybir.AluOpType.mult,
                        )
                        nc.tensor.matmul(pt[:], m[:], oh[:],
                                         start=(b == b0), stop=(b == b1 - 1))
                    nc.scalar.copy(dst_agg[:ch, w0:w0 + wlen], pt[:, :wlen])

            # ================= layer 1 =================
            seg_layer(x_full[:], DIN, aggT)

            for (c0, cw) in chunks:
                xt = nodew.tile([128, 512], F32, tag="xt")
                nc.sync.dma_start(xt[:, :cw], xT_loc[:, c0:c0 + cw])
                d1 = dpsum.tile([128, 512], F32, tag="big")
                nc.tensor.matmul(d1[:, :cw], W["Wl1T"][:], aggT[:, c0:c0 + cw],
                                 start=True, stop=False)
                nc.tensor.matmul(d1[:, :cw], W["Wr1T"][:], xt[:, :cw],
                                 start=False, stop=True)
                h1t = work.tile([128, 512], F32, tag="h1t")
                nc.scalar.activation(h1t[:, :cw], d1[:, :cw],
                                     mybir.ActivationFunctionType.Relu,
                                     bias=W["t1"][:], scale=W["s1"][:])
                nc.sync.dma_start(h1T_d[:, c0:c0 + cw], h1t[:, :cw])
                # g2 = (h1 @ Wl2.T) -> node-major table for layer-2 gathers
                g2p = dpsum.tile([64, 512], F32, tag="small")
                nc.tensor.matmul(g2p[:, :cw], W["Wl2T"][:], h1t[:, :cw],
                                 start=True, stop=True)
                g2s = work.tile([64, 512], F32, tag="g2s")
                nc.vector.tensor_copy(g2s[:, :cw], g2p[:, :cw])
                for j in range(0, cw, 128):
                    jw = min(128, cw - j)
                    tp = tpsum.tile([128, 64], F32, tag="tp")
                    nc.tensor.transpose(tp[:jw, :], g2s[:, j:j + jw],
                                        W["ident"][:64, :64])
                    tn = work.tile([128, 64], F32, tag="tn")
                    nc.vector.tensor_copy(tn[:jw, :], tp[:jw, :])
                    nc.sync.dma_start(g2_loc[c0 + j:c0 + j + jw, :], tn[:jw, :])

            nc.gpsimd.collective_compute(
                "AllGather", mybir.AluOpType.bypass,
                ins=[g2_loc[:]], outs=[g2_full[:]],
                replica_groups=[list(range(N_CORES))],
            )

            # ================= layer 2 =================
            seg_layer(g2_full[:], DOUT, aggT2)

            for (c0, cw) in chunks:
                h1t = nodew.tile([128, 512], F32, tag="xt")
                nc.sync.dma_start(h1t[:, :cw], h1T_d[:, c0:c0 + cw])
                zp = dpsum.tile([64, 512], F32, tag="small")
                nc.tensor.matmul(zp[:, :cw], W["Wr2T"][:], h1t[:, :cw],
                                 start=True, stop=True)
                zs = work.tile([64, 512], F32, tag="g2s")
                nc.vector.tensor_add(zs[:, :cw], zp[:, :cw],
                                     aggT2[:, c0:c0 + cw])
                zb = work.tile([64, 512], F32, tag="zb")
                nc.vector.tensor_scalar_add(zb[:, :cw], zs[:, :cw],
                                            W["bl2"][:])
                for j in range(0, cw, 128):
                    jw = min(128, cw - j)
                    tp = tpsum.tile([128, 64], F32, tag="tp")
                    nc.tensor.transpose(tp[:jw, :], zb[:, j:j + jw],
                                        W["ident"][:64, :64])
                    tn = work.tile([128, 64], F32, tag="tn")
                    nc.vector.tensor_copy(tn[:jw, :], tp[:jw, :])
                    nc.sync.dma_start(z_loc[c0 + j:c0 + j + jw, :], tn[:jw, :])

            nc.gpsimd.collective_compute(
                "AllGather", mybir.AluOpType.bypass,
                ins=[z_loc[:]], outs=[z_full[:]],
                replica_groups=[list(range(N_CORES))],
            )

            # ================= predictor =================
            for sb in range(NSB):
                combT = work.tile([128, 512], F32, tag="combT")
                for i in range(SB):
                    b = sb * SB + i
                    cb = msgs.tile([128, 128], F32, tag="cb")
                    nc.gpsimd.indirect_dma_start(
                        out=cb[:, 0:64], out_offset=None, in_=z_full[:],
                        in_offset=bass.IndirectOffsetOnAxis(
                            ap=psT[:, b:b + 1], axis=0),
                    )
                    nc.gpsimd.indirect_dma_start(
                        out=cb[:, 64:128], out_offset=None, in_=z_full[:],
                        in_offset=bass.IndirectOffsetOnAxis(
                            ap=pdT[:, b:b + 1], axis=0),
                    )
                    ctp = tpsum.tile([128, 128], F32, tag="tp")
                    nc.tensor.transpose(ctp[:], cb[:], W["ident"][:])
                    nc.vector.tensor_copy(combT[:, i * 128:(i + 1) * 128],
                                          ctp[:])
                eft = nodew.tile([32, 512], F32, tag="eft")
                nc.sync.dma_start(eft[:],
                                  efT[:, sb * 512:(sb + 1) * 512])
                em1 = dpsum.tile([64, 512], F32, tag="small")
                nc.tensor.matmul(em1[:], W["eW1T"][:], eft[:],
                                 start=True, stop=True)
                em1s = work.tile([64, 512], F32, tag="em1s")
                nc.scalar.activation(em1s[:], em1[:],
                                     mybir.ActivationFunctionType.Relu,
                                     bias=W["et"][:], scale=W["es"][:])
                em2 = dpsum.tile([32, 512], F32, tag="small")
                nc.tensor.matmul(em2[:], W["eW2T"][:], em1s[:],
                                 start=True, stop=True)
                em2s = work.tile([32, 512], F32, tag="em2s")
                nc.vector.tensor_scalar_add(em2s[:], em2[:], W["eb2"][:])
                u1 = dpsum.tile([128, 512], F32, tag="big")
                nc.tensor.matmul(u1[:], W["W1abT"][:], combT[:],
                                 start=True, stop=False)
                nc.tensor.matmul(u1[:], W["W1cT"][:], em2s[:],
                                 start=False, stop=True)
                u1s = work.tile([128, 512], F32, tag="u1s")
                nc.scalar.activation(u1s[:], u1[:],
                                     mybir.ActivationFunctionType.Relu,
                                     bias=W["pt1"][:], scale=W["ps1"][:])
                u2 = dpsum.tile([64, 512], F32, tag="small")
                nc.tensor.matmul(u2[:], W["W2pT"][:], u1s[:],
                                 start=True, stop=True)
                u2s = work.tile([64, 512], F32, tag="u2s")
                nc.scalar.activation(u2s[:], u2[:],
                                     mybir.ActivationFunctionType.Relu,
                                     bias=W["pt2"][:], scale=W["ps2"][:])
                uo = tpsum.tile([1, 512], F32, tag="tp")
                nc.tensor.matmul(uo[:], W["W3pT"][:], u2s[:],
                                 start=True, stop=True)
                uos = work.tile([1, 512], F32, tag="uos")
                nc.vector.tensor_scalar_add(uos[:], uo[:], W["pb3"][:])
                nc.sync.dma_start(
                    out[sb * 512:(sb + 1) * 512].rearrange("(p f) -> p f", p=1),
                    uos[:])

    nc.compile()
    return nc


def kernel(**inputs):
    from concourse import bass_utils

    in_maps, nblk, blk_starts = _prep_host(inputs)
    nc = _build(nblk, blk_starts)
    res = bass_utils.run_bass_kernel_spmd(
        nc, in_maps, core_ids=list(range(N_CORES)))
    outs = []
    for c in range(N_CORES):
        outs.append(res.results[c]["out"][:PPC])
    return np.concatenate(outs).astype(np.float32)

